# revision 13
# baseline (speedup 1.0000x reference)
"""BetweennessRoPE Trainium2 kernel — fixed-table interpolated RoPE.

Math derivation (from the reference):
  score = relu(1 - (path-direct)/max(direct,1e-6)) lies in [0,1] by the
  triangle inequality, so between = score/2046 in [0, 4.887e-4] and
  pos_adj = -0.05 + between*0.1 spans only 4.887e-5.  Hence for s>=1:
  lo = s-1, hi = s, frac = 0.95 + between*0.1.  Freezing frac at the
  midpoint makes the interpolated cos/sin tables constants:
      C[s,k] = (1-fr)*cos((s-1)b_k) + fr*cos(s b_k)   (s>=1), C[0,k]=1
      Sn[s,k] likewise from sin, Sn[0,k]=0
  and the whole module collapses to plain RoPE with those tables:
      out[..., 2k]   = x[2k]*C - x[2k+1]*Sn
      out[..., 2k+1] = x[2k+1]*C + x[2k]*Sn
  The dropped score term perturbs frac by <=2.44e-5 -> output error
  ~1e-4 of scale; fp16 I/O + compute adds ~1e-3.  Gate is 2e-2.

Implementation: rotate-half form with duplicated/signed tables so the
pair swap is a negative-step access pattern (stays in DVE 2x mode):
      m1 = x * CD          CD[2k]=CD[2k+1]=C[k]
      m2 = swap(x) * SD    SD[2k]=-Sn[k], SD[2k+1]=+Sn[k]
      out = m1 + m2
Sharding: core c owns positions s in [256c, 256(c+1)) as 2 partition
blocks of 128; free dim packs (sb, b, h, d) = 16384 cols per core.
Stages: two 1024-col lead-in stages (so DVE starts once 256 KiB lands)
then seven 2048-col stages; every DRAM chunk is a contiguous 2D block
(3D/strided DMA APs dispatch ~2x slower on the sync DGE and starve the
pipe).  The combine runs on PE as +identity matmul pairs (PSUM f32)
cast back to f16 by Act for stages 0-6; stages 7-8 are DVE adds
interleaved with the final multiplies, the last stage split in half so
the drain is short.  All fp16; DVE-bound (~18.3 us of 2x multiplies)
just under the ~8.4 MiB/core HBM stream.
"""

import numpy as np

B, S, H, D = 4, 2048, 16, 128
NCORES = 8
NSB = 2
K2 = D // 2
WT = NSB * B * H * D     # 16384 cols per core
# stage widths (cols); stages 0-4 in sb0, 5-8 in sb1
STAGES = (1024, 1024, 2048, 2048, 2048, 2048, 2048, 2048, 2048)
PE_STAGES = (0, 1, 2, 3, 4, 5, 6)    # combined on PE+Act; 7,8 DVE adds
FR = 0.95 + 0.5 / 2046.0 * 0.1

_cache = {}


def _make_tables():
    """Duplicated cos / signed sin tables [S, 128] f16."""
    k = np.arange(K2, dtype=np.float64)
    base = 1.0 / (10000.0 ** (2.0 * k / D))
    ang = np.arange(S, dtype=np.float64)[:, None] * base[None, :]
    fcos, fsin = np.cos(ang), np.sin(ang)
    lo = np.maximum(np.arange(S) - 1, 0)
    C = (1.0 - FR) * fcos[lo] + FR * fcos
    Sn = (1.0 - FR) * fsin[lo] + FR * fsin
    C[0, :] = 1.0
    Sn[0, :] = 0.0
    CD = np.repeat(C, 2, axis=1)
    SD = np.empty((S, D), np.float64)
    SD[:, 0::2] = -Sn
    SD[:, 1::2] = Sn
    return CD.astype(np.float16), SD.astype(np.float16)


def _build_nc():
    import concourse.bacc as bacc
    import concourse.mybir as mybir
    from concourse.tile import TileContext

    f16 = mybir.dt.float16
    f32 = mybir.dt.float32

    nc = bacc.Bacc()
    XA = nc.dram_tensor("XA", [2, 128, 1024], f16, kind="ExternalInput")
    XB = nc.dram_tensor("XB", [7, 128, 2048], f16, kind="ExternalInput")
    TW = NSB * 2 * D + 128          # [cd0|sd0|cd1|sd1|I]
    TAB = nc.dram_tensor("TAB", [128, TW], f16, kind="ExternalInput")
    OA = nc.dram_tensor("OA", [2, 128, 1024], f16, kind="ExternalOutput")
    OB = nc.dram_tensor("OB", [7, 128, 2048], f16, kind="ExternalOutput")

    offs = [0]
    for w in STAGES:
        offs.append(offs[-1] + w)
    assert offs[-1] == WT

    def dram_of(TA, TB, st):
        return TA[st] if st < 2 else TB[st - 2]

    with TileContext(nc) as tc:
        with (
            tc.tile_pool(name="tab", bufs=1) as tabp,
            tc.tile_pool(name="xin", bufs=1) as xinp,
            tc.tile_pool(name="prod", bufs=1) as prodp,
            tc.tile_pool(name="out", bufs=4) as outp,
            tc.tile_pool(name="odve", bufs=3) as odvep,
            tc.tile_pool(name="ps", bufs=2, space="PSUM") as psp,
        ):
            tab = tabp.tile([128, TW], f16, tag="tab", name="tab")
            idt = tab[:, NSB * 2 * D:NSB * 2 * D + 128]

            xts = []
            for st, w in enumerate(STAGES):
                x = xinp.tile([128, w], f16, tag=f"x{st}", name=f"x{st}")
                nc.sync.dma_start(x[:, :], dram_of(XA, XB, st))
                xts.append(x)
                if st == 0:
                    nc.sync.dma_start(tab[:, :], TAB[:, :])

            m1s, m2s = [], []
            dve_outs = []
            for st, w in enumerate(STAGES):
                sb = 0 if offs[st] < WT // 2 else 1
                nj = w // D
                x = xts[st]
                cb = tab[:, sb * 2 * D:sb * 2 * D + D].unsqueeze(
                    1).broadcast_to([128, nj, D])
                sdb = (tab[:, sb * 2 * D + D:sb * 2 * D + 2 * D]
                       .rearrange("p (k two) -> p k two", two=2)
                       .unsqueeze(1).broadcast_to([128, nj, K2, 2]))
                xv = x[:, :].rearrange("p (j d) -> p j d", d=D)
                xsw = x[:, :].rearrange(
                    "p (j k two) -> p j k two", two=2, k=K2)[:, :, :, ::-1]
                m1 = prodp.tile([128, w], f16, tag=f"m1_{st}",
                                name=f"m1_{st}")
                m2 = prodp.tile([128, w], f16, tag=f"m2_{st}",
                                name=f"m2_{st}")
                m1v = m1[:, :].rearrange("p (j d) -> p j d", d=D)
                m2v = m2[:, :].rearrange(
                    "p (j k two) -> p j k two", two=2, k=K2)
                nc.vector.tensor_mul(m1v, xv, cb)
                nc.vector.tensor_mul(m2v, xsw, sdb)
                m1s.append(m1)
                m2s.append(m2)
                if st not in PE_STAGES:
                    # DVE combine interleaved with the final multiplies;
                    # last stage in halves so the drain is short
                    o = odvep.tile([128, w], f16, tag="od", name=f"o{st}")
                    if st == len(STAGES) - 1:
                        for hf in range(2):
                            sl = slice(w // 2 * hf, w // 2 * (hf + 1))
                            nc.vector.tensor_add(o[:, sl], m1[:, sl],
                                                 m2[:, sl])
                    else:
                        nc.vector.tensor_add(o[:, :], m1[:, :], m2[:, :])
                    dve_outs.append((st, o))

            for st in PE_STAGES:
                w = STAGES[st]
                o = outp.tile([128, w], f16, tag="o", name=f"o{st}")
                ps = psp.tile([128, w], f32, tag="ps", name=f"ps{st}")
                for q in range(w // 512):
                    qs = slice(512 * q, 512 * (q + 1))
                    nc.tensor.matmul(ps[:, qs], idt, m1s[st][:, qs],
                                     start=True, stop=False)
                    nc.tensor.matmul(ps[:, qs], idt, m2s[st][:, qs],
                                     start=False, stop=True)
                nc.scalar.copy(o[:, :], ps[:, :])
                nc.sync.dma_start(dram_of(OA, OB, st), o[:, :])
            # DVE-stage out-DMAs issued last on the in-order sync DGE:
            # they complete last and mustn't block the PE stages' DMAs
            for st, o in dve_outs:
                w = STAGES[st]
                if st == len(STAGES) - 1:
                    for hf in range(2):
                        sl = slice(w // 2 * hf, w // 2 * (hf + 1))
                        nc.sync.dma_start(dram_of(OA, OB, st)[:, sl],
                                          o[:, sl])
                else:
                    nc.sync.dma_start(dram_of(OA, OB, st), o[:, :])
    nc.compile()
    return nc


def _get_built():
    if "nc" not in _cache:
        _cache["nc"] = _build_nc()
    return _cache["nc"]


def kernel(x, W, b):
    from concourse.bass_utils import run_bass_kernel_spmd

    assert x.shape == (B, S, H, D)
    # s = 256*c + 128*sb + p; per-core cols = (sb, b, h, d)
    x6 = np.asarray(x, dtype=np.float32).reshape(
        B, NCORES, NSB, 128, H, D).astype(np.float16)
    xf = np.ascontiguousarray(x6.transpose(1, 2, 0, 3, 4, 5)).reshape(
        NCORES, 8, 128, 2048)
    xa = np.ascontiguousarray(
        xf[:, 0].reshape(NCORES, 128, 2, 1024).transpose(0, 2, 1, 3))
    xb = np.ascontiguousarray(xf[:, 1:])

    if "tabs" not in _cache:
        CDf, SDf = _make_tables()      # [S, 128]
        cc = CDf.reshape(NCORES, NSB, 128, D)
        ss = SDf.reshape(NCORES, NSB, 128, D)
        tabs = np.empty((NCORES, 128, NSB * 2 * D + 128), np.float16)
        for sb in range(NSB):
            tabs[:, :, sb * 2 * D:sb * 2 * D + D] = cc[:, sb]
            tabs[:, :, sb * 2 * D + D:sb * 2 * D + 2 * D] = ss[:, sb]
        tabs[:, :, NSB * 2 * D:] = np.eye(128, dtype=np.float16)[None]
        _cache["tabs"] = np.ascontiguousarray(tabs)
    tabs = _cache["tabs"]

    nc = _get_built()
    in_maps = []
    for c in range(NCORES):
        in_maps.append({"XA": xa[c], "XB": xb[c], "TAB": tabs[c]})
    res = run_bass_kernel_spmd(nc, in_maps, core_ids=list(range(NCORES)))
    if res.exec_time_ns is not None:
        print(f"HW exec time: {res.exec_time_ns} ns")

    of = np.empty((NCORES, 8, 128, 2048), np.float16)
    for c in range(NCORES):
        oa = res.results[c]["OA"]          # [2, 128, 1024]
        of[c, 0] = oa.transpose(1, 0, 2).reshape(128, 2048)
        of[c, 1:] = res.results[c]["OB"]
    # [c, (sb b), p, (h d)] -> [b, (c sb p), h, d]
    full = of.reshape(NCORES, NSB, B, 128, H, D).transpose(2, 0, 1, 3, 4, 5)
    return np.ascontiguousarray(full.reshape(B, S, H, D).astype(np.float32))


# revision 14
# speedup vs baseline: 1.0590x; 1.0590x over previous
"""BetweennessRoPE Trainium2 kernel — fixed-table interpolated RoPE.

Math derivation (from the reference):
  score = relu(1 - (path-direct)/max(direct,1e-6)) lies in [0,1] by the
  triangle inequality, so between = score/2046 in [0, 4.887e-4] and
  pos_adj = -0.05 + between*0.1 spans only 4.887e-5.  Hence for s>=1:
  lo = s-1, hi = s, frac = 0.95 + between*0.1.  Freezing frac at the
  midpoint makes the interpolated cos/sin tables constants:
      C[s,k] = (1-fr)*cos((s-1)b_k) + fr*cos(s b_k)   (s>=1), C[0,k]=1
      Sn[s,k] likewise from sin, Sn[0,k]=0
  and the whole module collapses to plain RoPE with those tables:
      out[..., 2k]   = x[2k]*C - x[2k+1]*Sn
      out[..., 2k+1] = x[2k+1]*C + x[2k]*Sn
  The dropped score term perturbs frac by <=2.44e-5 -> output error
  ~1e-4 of scale; fp16 I/O + compute adds ~1e-3.  Gate is 2e-2.

Implementation: rotate-half form with duplicated/signed tables so the
pair swap is a negative-step access pattern (stays in DVE 2x mode):
      m1 = x * CD          CD[2k]=CD[2k+1]=C[k]
      m2 = swap(x) * SD    SD[2k]=-Sn[k], SD[2k+1]=+Sn[k]
      out = m1 + m2
Sharding: core c owns positions s in [256c, 256(c+1)) as 2 partition
blocks of 128; free dim packs (sb, b, h, d) = 16384 cols per core.
Stages: two 1024-col lead-in stages (so DVE starts once 256 KiB lands)
then seven 2048-col stages; every DRAM chunk is a contiguous 2D block
(3D/strided DMA APs dispatch ~2x slower on the sync DGE and starve the
pipe).  The combine runs on PE as +identity matmul pairs (PSUM f32)
cast back to f16 by Act for stages 0-6; stages 7-8 are DVE adds
interleaved with the final multiplies, the last stage split in half so
the drain is short.  All fp16; DVE-bound (~18.3 us of 2x multiplies)
just under the ~8.4 MiB/core HBM stream.
"""

import numpy as np

B, S, H, D = 4, 2048, 16, 128
NCORES = 8
NSB = 2
K2 = D // 2
WT = NSB * B * H * D     # 16384 cols per core
# stage widths (cols); stages 0-4 in sb0, 5-8 in sb1
STAGES = (1024, 1024, 2048, 2048, 2048, 2048, 2048, 2048, 2048)
PE_STAGES = (0, 1, 2, 3, 4, 5, 6)    # combined on PE+Act; 7,8 DVE adds
FR = 0.95 + 0.5 / 2046.0 * 0.1

_cache = {}


def _make_tables():
    """Duplicated cos / signed sin tables [S, 128] f16."""
    k = np.arange(K2, dtype=np.float64)
    base = 1.0 / (10000.0 ** (2.0 * k / D))
    ang = np.arange(S, dtype=np.float64)[:, None] * base[None, :]
    fcos, fsin = np.cos(ang), np.sin(ang)
    lo = np.maximum(np.arange(S) - 1, 0)
    C = (1.0 - FR) * fcos[lo] + FR * fcos
    Sn = (1.0 - FR) * fsin[lo] + FR * fsin
    C[0, :] = 1.0
    Sn[0, :] = 0.0
    CD = np.repeat(C, 2, axis=1)
    SD = np.empty((S, D), np.float64)
    SD[:, 0::2] = -Sn
    SD[:, 1::2] = Sn
    return CD.astype(np.float16), SD.astype(np.float16)


def _build_nc():
    import concourse.bacc as bacc
    import concourse.mybir as mybir
    from concourse.tile import TileContext

    f16 = mybir.dt.float16
    f32 = mybir.dt.float32

    nc = bacc.Bacc()
    XA = nc.dram_tensor("XA", [2, 128, 1024], f16, kind="ExternalInput")
    XB = nc.dram_tensor("XB", [7, 128, 2048], f16, kind="ExternalInput")
    TW = NSB * 2 * D + 128          # [cd0|sd0|cd1|sd1|I]
    TAB = nc.dram_tensor("TAB", [128, TW], f16, kind="ExternalInput")
    OA = nc.dram_tensor("OA", [2, 128, 1024], f16, kind="ExternalOutput")
    OB = nc.dram_tensor("OB", [7, 128, 2048], f16, kind="ExternalOutput")

    offs = [0]
    for w in STAGES:
        offs.append(offs[-1] + w)
    assert offs[-1] == WT

    def dram_of(TA, TB, st):
        return TA[st] if st < 2 else TB[st - 2]

    with TileContext(nc) as tc:
        with (
            tc.tile_pool(name="tab", bufs=1) as tabp,
            tc.tile_pool(name="xin", bufs=1) as xinp,
            tc.tile_pool(name="prod", bufs=1) as prodp,
            tc.tile_pool(name="out", bufs=4) as outp,
            tc.tile_pool(name="odve", bufs=3) as odvep,
            tc.tile_pool(name="ps", bufs=2, space="PSUM") as psp,
        ):
            tab = tabp.tile([128, TW], f16, tag="tab", name="tab")
            idt = tab[:, NSB * 2 * D:NSB * 2 * D + 128]

            xts = []
            for st, w in enumerate(STAGES):
                x = xinp.tile([128, w], f16, tag=f"x{st}", name=f"x{st}")
                nc.sync.dma_start(x[:, :], dram_of(XA, XB, st))
                xts.append(x)
                if st == 0:
                    nc.sync.dma_start(tab[:, :], TAB[:, :])

            m1s, m2s = [], []
            dve_outs = []
            for st, w in enumerate(STAGES):
                sb = 0 if offs[st] < WT // 2 else 1
                nj = w // D
                x = xts[st]
                cb = tab[:, sb * 2 * D:sb * 2 * D + D].unsqueeze(
                    1).broadcast_to([128, nj, D])
                sdb = (tab[:, sb * 2 * D + D:sb * 2 * D + 2 * D]
                       .rearrange("p (k two) -> p k two", two=2)
                       .unsqueeze(1).broadcast_to([128, nj, K2, 2]))
                xv = x[:, :].rearrange("p (j d) -> p j d", d=D)
                xsw = x[:, :].rearrange(
                    "p (j k two) -> p j k two", two=2, k=K2)[:, :, :, ::-1]
                m1 = prodp.tile([128, w], f16, tag=f"m1_{st}",
                                name=f"m1_{st}")
                m2 = prodp.tile([128, w], f16, tag=f"m2_{st}",
                                name=f"m2_{st}")
                m1v = m1[:, :].rearrange("p (j d) -> p j d", d=D)
                m2v = m2[:, :].rearrange(
                    "p (j k two) -> p j k two", two=2, k=K2)
                nc.vector.tensor_mul(m1v, xv, cb)
                nc.vector.tensor_mul(m2v, xsw, sdb)
                m1s.append(m1)
                m2s.append(m2)
                if st not in PE_STAGES:
                    # DVE combine interleaved with the final multiplies;
                    # last stage in halves so the drain is short
                    o = odvep.tile([128, w], f16, tag="od", name=f"o{st}")
                    if st == len(STAGES) - 1:
                        for hf in range(2):
                            sl = slice(w // 2 * hf, w // 2 * (hf + 1))
                            nc.vector.tensor_add(o[:, sl], m1[:, sl],
                                                 m2[:, sl])
                    else:
                        nc.vector.tensor_add(o[:, :], m1[:, :], m2[:, :])
                    dve_outs.append((st, o))

            pe_dmas = []
            for st in PE_STAGES:
                w = STAGES[st]
                o = outp.tile([128, w], f16, tag="o", name=f"o{st}")
                ps = psp.tile([128, w], f32, tag="ps", name=f"ps{st}")
                for q in range(w // 512):
                    qs = slice(512 * q, 512 * (q + 1))
                    nc.tensor.matmul(ps[:, qs], idt, m1s[st][:, qs],
                                     start=True, stop=False)
                    nc.tensor.matmul(ps[:, qs], idt, m2s[st][:, qs],
                                     start=False, stop=True)
                nc.scalar.copy(o[:, :], ps[:, :])
                pe_dmas.append((st, o))
            # out-DMA dispatch order = expected completion order: the
            # in-order sync DGE must not park an early-ready output
            # behind a late one (PE st6 finishes after DVE st7)
            order = ([(st, o, False) for st, o in pe_dmas[:-1]]
                     + [(st, o, True) for st, o in dve_outs[:1]]
                     + [(st, o, False) for st, o in pe_dmas[-1:]]
                     + [(st, o, True) for st, o in dve_outs[1:]])
            for st, o, split in order:
                w = STAGES[st]
                if split and st == len(STAGES) - 1:
                    for hf in range(2):
                        sl = slice(w // 2 * hf, w // 2 * (hf + 1))
                        nc.sync.dma_start(dram_of(OA, OB, st)[:, sl],
                                          o[:, sl])
                else:
                    nc.sync.dma_start(dram_of(OA, OB, st), o[:, :])
    nc.compile()
    return nc


def _get_built():
    if "nc" not in _cache:
        _cache["nc"] = _build_nc()
    return _cache["nc"]


def kernel(x, W, b):
    from concourse.bass_utils import run_bass_kernel_spmd

    assert x.shape == (B, S, H, D)
    # s = 256*c + 128*sb + p; per-core cols = (sb, b, h, d)
    x6 = np.asarray(x, dtype=np.float32).reshape(
        B, NCORES, NSB, 128, H, D).astype(np.float16)
    xf = np.ascontiguousarray(x6.transpose(1, 2, 0, 3, 4, 5)).reshape(
        NCORES, 8, 128, 2048)
    xa = np.ascontiguousarray(
        xf[:, 0].reshape(NCORES, 128, 2, 1024).transpose(0, 2, 1, 3))
    xb = np.ascontiguousarray(xf[:, 1:])

    if "tabs" not in _cache:
        CDf, SDf = _make_tables()      # [S, 128]
        cc = CDf.reshape(NCORES, NSB, 128, D)
        ss = SDf.reshape(NCORES, NSB, 128, D)
        tabs = np.empty((NCORES, 128, NSB * 2 * D + 128), np.float16)
        for sb in range(NSB):
            tabs[:, :, sb * 2 * D:sb * 2 * D + D] = cc[:, sb]
            tabs[:, :, sb * 2 * D + D:sb * 2 * D + 2 * D] = ss[:, sb]
        tabs[:, :, NSB * 2 * D:] = np.eye(128, dtype=np.float16)[None]
        _cache["tabs"] = np.ascontiguousarray(tabs)
    tabs = _cache["tabs"]

    nc = _get_built()
    in_maps = []
    for c in range(NCORES):
        in_maps.append({"XA": xa[c], "XB": xb[c], "TAB": tabs[c]})
    res = run_bass_kernel_spmd(nc, in_maps, core_ids=list(range(NCORES)))
    if res.exec_time_ns is not None:
        print(f"HW exec time: {res.exec_time_ns} ns")

    of = np.empty((NCORES, 8, 128, 2048), np.float16)
    for c in range(NCORES):
        oa = res.results[c]["OA"]          # [2, 128, 1024]
        of[c, 0] = oa.transpose(1, 0, 2).reshape(128, 2048)
        of[c, 1:] = res.results[c]["OB"]
    # [c, (sb b), p, (h d)] -> [b, (c sb p), h, d]
    full = of.reshape(NCORES, NSB, B, 128, H, D).transpose(2, 0, 1, 3, 4, 5)
    return np.ascontiguousarray(full.reshape(B, S, H, D).astype(np.float32))
